# revision 67
# baseline (speedup 1.0000x reference)
"""Trainium2 Bass kernel for nn_Architecture_59760174956735 (dense_mlp).

Pure data parallel over 8 NeuronCores: batch 32768 -> 8 x 4096 rows,
weights replicated; no collectives. Host prep transposes x to
feature-major fp8-e4m3 (f = z*16 + c as [partition, tile, chunk, col]),
so no on-chip transpose is needed and the x DMA is 8.4 MB/core.

Per core, a software pipeline over 8 N-tiles of 512 batch columns:
step s issues the locally-connected layer for tile s (fp8, 31 M=32
matmuls 4-way col-rotated via tile_position), W1 for tile s-1 and W2
for tile s-2 (both fp8 DoubleRow: K packed 2/cell, rhs [128,2,N] is the
natural chunk-pair layout), plus interleaved pair-tail steps computing
the five tiny tail layers for two tiles packed per PSUM bank at
partition bases 0/64 (tail weights zero-padded to M=64). PSUM->SBUF
epilogues fuse bias+ReLU, alternating Activation / Vector engines.
The packed tail-weight tensor carries only W3..Ws4 (98 KB) and weight
DMAs are ordered by first use so the x stream starts early; each
pair's two output rows go out as one strided DMA on the Activation
HWDGE ring, and x-tile DMAs alternate between the sync and Activation
rings so both HWDGE queue rows feed the SDMA engines; the repeat
loop uses For_i(staggered_reset=True) with sixteen body unrolls per
loop iteration, so the all-engine barrier fires every sixteenth pass; each
pass's eleven trailing tail steps are deferred into the NEXT pass's
early steps (two per step, drained first), so their latency-bound
chain links never park at the PE queue head in front of the next
pass's ready matmuls (-6.7us, the largest single win). The
cb=3 PSUM bank's pad rows (96:128) are zeroed once before the loop
instead of by a per-tile dummy matmul, and W1's second output group is
sliced to its real 91 columns with the h1p1 bank's pad rows pre-zeroed
the same way (start=True clears has_written bits, not data, so the
zeros persist for every epilogue read). Within each step, ready work
(W2, tail, W1) is emitted before the DMA-dependent local matmuls so
pass-boundary DMA waits never block the PE queue head.
Quantization error of the fp8
inputs/weights averages out through the deep contraction; measured
output rel err vs the f32 reference ~6.6e-4.
"""
import numpy as np
import ml_dtypes
from contextlib import ExitStack

from concourse import bacc, tile, mybir
from concourse.bass_utils import run_bass_kernel_spmd

BF16 = ml_dtypes.bfloat16
FP8 = ml_dtypes.float8_e4m3

BF = mybir.dt.bfloat16
F8 = mybir.dt.float8e4
F32 = mybir.dt.float32
Relu = mybir.ActivationFunctionType.Relu
Ident = mybir.ActivationFunctionType.Identity
ADD = mybir.AluOpType.add
MAX = mybir.AluOpType.max

NCORES = 8
BATCH = 32768
BC = BATCH // NCORES
NT = 512
NTILES = BC // NT

L, NHF, F1, S1_, NCH, NZ = 15, 32, 16, 8, 16, 128
H1, H2, H3, NF = 219, 100, 45, 21


L, NHF, F1, S1, NCH, NZ = 15, 32, 16, 8, 16, 128
H1, H2, H3, NF = 219, 100, 45, 21

# column offsets inside the packed tail-weight array wb [128, 384].
# Tail layers are zero-padded to M=64 so pair-packed matmuls write the full
# 64-partition range (epilogues may then read [0:128) without stale bytes).
OFF_W3 = 0            # 45 used of 64
OFF_W4 = 64           # 21 used of 64
OFF_S1 = 128          # 20 used of 64
OFF_S2 = 192          # 20 used of 64
OFF_S3 = 256          # 20 used of 64
OFF_S4 = 320          # 1 used of 64
WB_COLS = 384


def pack_x_shard(xs: np.ndarray, group: int = 512, dtype=FP8) -> np.ndarray:
    """(Bc, 16, 128) f32 -> xq2[p, g, q, j] where feature f = 128q + p
    (f = z*16+c) and batch b = g*group + j. Per-partition data for one batch
    group is contiguous for descriptor-efficient DMA."""
    Bc = xs.shape[0]
    xt = xs.astype(dtype).transpose(2, 1, 0)           # [z, c, b]
    xt = xt.reshape(16, 8, NCH, Bc)                    # [q, dz, c, b]
    xq = xt.transpose(1, 2, 0, 3).reshape(128, 16, Bc)  # [p, q, b]
    xq2 = xq.reshape(128, 16, Bc // group, group).transpose(0, 2, 1, 3)
    return np.ascontiguousarray(xq2)                   # [p, g, q, j]


def pack_local_fp8(W_local) -> np.ndarray:
    """Local-layer weights in fp8-e4m3: wl8[p, (2l+m)*32+o] = Wt[l, 128m+p, o]."""
    T = W_local.reshape(L, NHF, NCH, F1)               # [l, o, c, k]
    Wt = T.transpose(0, 3, 2, 1).reshape(L, 256, NHF)  # [l, k*16+c, o]
    wl = Wt.reshape(L, 2, 128, NHF).transpose(2, 0, 1, 3).reshape(128, 960)
    out = np.zeros((128, 1024), np.float32)  # cols 960:1024 all-zero block
    out[:, :960] = wl
    return out.astype(FP8)


def pack_weights(W3, W4, Ws1, Ws2, Ws3, Ws4) -> np.ndarray:
    wb = np.zeros((128, WB_COLS), dtype=np.float32)
    wb[:100, OFF_W3:OFF_W3 + H3] = W3.T
    # tail weights replicated at partition base 64 for position-rotated MMs
    for base in (0, 64):
        wb[base:base + H3, OFF_W4:OFF_W4 + 21] = W4.T
        wb[base:base + 21, OFF_S1:OFF_S1 + 20] = Ws1.T
        wb[base:base + 20, OFF_S2:OFF_S2 + 20] = Ws2.T
        wb[base:base + 20, OFF_S3:OFF_S3 + 20] = Ws3.T
        wb[base:base + 20, OFF_S4:OFF_S4 + 1] = Ws4.T
    return wb.astype(BF16)


def pack_biases(b_local, b1, b2, b3, b4, bs1, bs2, bs3, bs4) -> np.ndarray:
    bb = np.zeros((128, 13), dtype=np.float32)
    bl = b_local.reshape(480)
    for c in range(4):
        n = min(128, 480 - c * 128)
        bb[:n, c] = bl[c * 128:c * 128 + n]
    bb[:128, 4] = b1[:128]
    bb[:91, 5] = b1[128:]
    bb[:100, 6] = b2
    for base in (0, 64):
        bb[base:base + H3, 7] = b3
        bb[base:base + 21, 8] = b4
        bb[base:base + 20, 9] = bs1
        bb[base:base + 20, 10] = bs2
        bb[base:base + 20, 11] = bs3
        bb[base:base + 1, 12] = bs4
    return bb


def pack_w1_dr(W1) -> np.ndarray:
    """W1 for fp8 DoubleRow: wd1[p, pair, i, o] = W1T_pad[128*(2*pair+i)+p, o],
    M padded to 224 so the i-stride (224 B) is 16-aligned."""
    w1t = np.zeros((512, 256), np.float32)
    w1t[:480, :H1] = W1.T
    return np.ascontiguousarray(
        w1t.reshape(2, 2, 128, 256).transpose(2, 0, 1, 3)).astype(FP8)


def pack_w2_dr(W2) -> np.ndarray:
    """W2 for fp8 DoubleRow: wd2[p, i, o] = W2T_pad[128*i+p, o], M pad 112."""
    w2t = np.zeros((256, 112), np.float32)
    w2t[:219, :H2] = W2.T
    return np.ascontiguousarray(
        w2t.reshape(2, 128, 112).transpose(1, 0, 2)).astype(FP8)


STAGES = {"dma": 0, "local": 1, "w1": 2, "w2": 3, "notail": 3, "full": 3}

# tail steps emitted after W2 of tile t2 (pair p ready once t2 >= 2p+1)
TAIL_AT = {2: [(0, 0)], 3: [(0, 1)],
           4: [(0, 2), (1, 0)], 5: [(0, 3), (1, 1)],
           6: [(0, 4), (1, 2), (2, 0)],
           7: [(0, 5), (1, 3), (2, 1), (3, 0)]}
TAIL_TRAILING = [(1, 4), (2, 2), (3, 1), (1, 5), (2, 3), (3, 2),
                 (2, 4), (3, 3), (2, 5), (3, 4), (3, 5)]
# previous pass's trailing steps drained inside the next pass, oldest
# first, two per main step (chain links get a full step of epi slack)
TRAIL_IN = {0: [(1, 4), (2, 2)], 1: [(3, 1), (1, 5)],
            2: [(2, 3), (3, 2)], 3: [(2, 4), (3, 3)],
            4: [(2, 5), (3, 4)], 5: [(3, 5)]}
# alternative placements of the deferred steps
TRAIL_IN_SPREAD = {0: [(1, 4)], 1: [(2, 2)], 2: [(3, 1)], 3: [(1, 5)],
                   4: [(2, 3)], 5: [(3, 2)], 6: [(2, 4)], 7: [(3, 3)],
                   8: [(2, 5), (3, 4)], 9: [(3, 5)]}
TRAIL_IN_LATE = {2: [(1, 4), (2, 2)], 3: [(3, 1), (1, 5)],
                 4: [(2, 3), (3, 2)], 5: [(2, 4), (3, 3)],
                 6: [(2, 5), (3, 4)], 7: [(3, 5)]}
# half-N (256-col) trailing: deep chains of pairs 2/3 advance in half
# steps on alternating engines, roughly halving per-link latency
TAIL_TRAILING_HALF = [
    (1, 4, 0), (1, 4, 1), (2, 2, 0), (2, 2, 1), (3, 1, 0), (3, 1, 1),
    (1, 5, 0), (1, 5, 1), (2, 3, 0), (2, 3, 1), (3, 2, 0), (3, 2, 1),
    (2, 4, 0), (2, 4, 1), (3, 3, 0), (3, 3, 1),
    (2, 5, 0), (2, 5, 1), (3, 4, 0), (3, 4, 1),
    (3, 5, 0), (3, 5, 1),
]


def build_nc(repeat=1, hw_loop=False, mode="full", out_ring="scalar",
             x0split=False, order="rev", half_trail=False, warmup=0,
             xbufs=3, stag=True, outmerge=True, unroll2=16,
             nodummy=True, w1slim=True, xring2=True, order2=False,
             defer=True, trailin="two"):
    last_stage = STAGES[mode]
    nc = bacc.Bacc(None, target_bir_lowering=False)
    xq_ext = nc.declare_dram_parameter(
        "xq", [128, NTILES, 16, NT], F8, isOutput=False)
    wb_ext = nc.declare_dram_parameter("wb", [128, WB_COLS], BF, isOutput=False)
    wl8_ext = nc.declare_dram_parameter("wl8", [128, 1024], F8, isOutput=False)
    wd1_ext = nc.declare_dram_parameter("wd1", [128, 2, 2, 256], F8,
                                        isOutput=False)
    wd2_ext = nc.declare_dram_parameter("wd2", [128, 2, 112], F8,
                                        isOutput=False)
    bb_ext = nc.declare_dram_parameter("bb", [128, 13], F32, isOutput=False)
    out_ext = nc.declare_dram_parameter("out", [1, BC], F32, isOutput=True)

    with tile.TileContext(nc) as tc, ExitStack() as ctx:
        wpool = ctx.enter_context(tc.tile_pool(name="w", bufs=1))
        xpool = ctx.enter_context(tc.tile_pool(name="x", bufs=xbufs))
        hpool = ctx.enter_context(tc.tile_pool(name="h", bufs=2))
        apool = ctx.enter_context(tc.tile_pool(name="a", bufs=1))
        opool = ctx.enter_context(tc.tile_pool(name="o", bufs=4))

        wb = wpool.tile([128, WB_COLS], BF, tag="wb")
        bb = wpool.tile([128, 13], F32, tag="bb")
        wl8 = wpool.tile([128, 1024], F8, tag="wl8")
        wd1 = wpool.tile([128, 2, 2, 256], F8, tag="wd1")
        wd2 = wpool.tile([128, 2, 112], F8, tag="wd2")
        # order by first use: local weights, then W1/W2, then tail
        nc.sync.dma_start(wl8[:], wl8_ext[:])
        nc.sync.dma_start(bb[:], bb_ext[:])
        nc.sync.dma_start(wd1[:], wd1_ext[:])
        nc.sync.dma_start(wd2[:], wd2_ext[:])
        nc.sync.dma_start(wb[:], wb_ext[:])

        def epilogue(i, out_ap, in_ap, bias_ap, relu=True):
            if not relu:
                nc.scalar.activation(out_ap, in_ap, Ident, bias=bias_ap)
            elif i % 2 == 0:
                nc.scalar.activation(out_ap, in_ap, Relu, bias=bias_ap)
            else:
                nc.vector.tensor_scalar(out_ap, in_ap, bias_ap, 0.0,
                                        op0=ADD, op1=MAX)

        out_eng = nc.sync if out_ring == "sync" else nc.scalar

        def out_probe(t, src_ap):
            """Stripped-mode output: 1-row copy + DMA so work stays live."""
            osb = opool.tile([1, NT], F32, tag="osb")
            nc.vector.tensor_copy(osb[:1, :], src_ap)
            out_eng.dma_start(out_ext[0:1, t * NT:(t + 1) * NT], osb[:1, :])

        with tc.tile_pool(name="p0", bufs=1, space="PSUM") as pp0, \
             tc.tile_pool(name="p1", bufs=1, space="PSUM") as pp1, \
             tc.tile_pool(name="pb", bufs=2, space="PSUM") as ppb:

            def stage_local(t, h0s):
                xsb = xpool.tile([128, 16, NT], F8, tag="xsb")
                xeng = nc.scalar if (xring2 and t % 2) else nc.sync
                if x0split and t == 0:
                    for lo, hi in ((0, 5), (5, 9), (9, 13), (13, 16)):
                        xeng.dma_start(xsb[:, lo:hi, :],
                                       xq_ext[:, 0, lo:hi, :])
                else:
                    xeng.dma_start(xsb[:], xq_ext[:, t, :, :])
                if last_stage == 0:
                    out_probe(t, xsb[:1, 0, :])
                    return
                h0 = hpool.tile([128, 4, NT], F8, tag="h0")
                for cb in range(4):
                    h0p = pp0.tile([128, NT], F32, tag=f"h0p{cb}")
                    nblk = 4 if cb < 3 else 3
                    if cb == 3 and not nodummy:
                        # zero dummy block so rows 96:128 are written
                        nc.tensor.matmul(h0p[96:128, :], wl8[:, 960:992],
                                         xsb[:, 15, :], start=True, stop=True,
                                         tile_position=(0, 96),
                                         skip_group_check=True)
                    for m in (0, 1):
                        for i in range(nblk):
                            l = cb * 4 + i
                            po = 32 * i
                            nc.tensor.matmul(
                                h0p[po:po + 32, :],
                                wl8[:, (2 * l + m) * 32:
                                       (2 * l + m + 1) * 32],
                                xsb[:, l + m, :],
                                start=(m == 0), stop=(m == 1),
                                tile_position=(0, po),
                                skip_group_check=True,
                            )
                    epilogue(t + cb, h0[:, cb, :], h0p[:, :],
                             bb[:, cb:cb + 1])
                h0s[t] = h0
                if last_stage == 1:
                    out_probe(t, h0[:1, 0, :])

            def stage_w1(t, h0s, h1s):
                h0 = h0s.pop(t)
                h1 = hpool.tile([128, 2, NT], F8, tag="h1")
                for mo in range(2):
                    h1p = pp1.tile([128, NT], F32, tag=f"h1p{mo}")
                    mw = 91 if (w1slim and mo == 1) else 128
                    for pair in (0, 1):
                        nc.tensor.matmul(
                            h1p[:mw, :],
                            wd1[:, pair, :, 128 * mo:128 * mo + mw],
                            h0[:, 2 * pair:2 * pair + 2, :],
                            start=(pair == 0), stop=(pair == 1),
                            perf_mode=mybir.MatmulPerfMode.DoubleRow,
                        )
                    epilogue(t + mo, h1[:, mo, :], h1p[:, :],
                             bb[:, 4 + mo:5 + mo])
                h1s[t] = h1
                if last_stage == 2:
                    out_probe(t, h1[:1, 0, :])

            def stage_w2(t, h1s, h2all):
                h1 = h1s.pop(t)
                h2p = ppb.tile([128, NT], F32, tag="pb")
                nc.tensor.matmul(
                    h2p[:100, :],
                    wd2[:, :, :100],
                    h1[:, 0:2, :],
                    start=True, stop=True,
                    perf_mode=mybir.MatmulPerfMode.DoubleRow,
                )
                epilogue(t, h2all[:100, t, :], h2p[:100, :], bb[:100, 6:7])
                if mode == "w2":
                    out_probe(t, h2all[:1, t, :])

            def make_tail(h2all):
                h3a = apool.tile([128, 4, NT], BF, tag="h3a")
                h4a = apool.tile([128, 4, NT], BF, tag="h4a")
                s1a = apool.tile([128, 4, NT], BF, tag="s1a")
                s2a = apool.tile([128, 4, NT], BF, tag="s2a")
                s3a = apool.tile([128, 4, NT], BF, tag="s3a")
                chain = [(None, h3a, 100, OFF_W3, 7),
                         (h3a, h4a, 45, OFF_W4, 8),
                         (h4a, s1a, 21, OFF_S1, 9),
                         (s1a, s2a, 20, OFF_S2, 10),
                         (s2a, s3a, 20, OFF_S3, 11),
                         (s3a, None, 20, OFF_S4, 12)]

                s4osb = {}

                def do_step(p, li, n=None):
                    src, dst, K, woff, bcol = chain[li]
                    sl = (slice(0, NT) if n is None
                          else slice(n * 256, (n + 1) * 256))
                    pt = ppb.tile([128, NT], F32, tag="pb")
                    for s in (0, 1):
                        b = 64 * s
                        if src is None:
                            lhsT = wb[:100, woff:woff + 64]
                            rhs = h2all[:100, 2 * p + s, sl]
                            pos = (0, b)
                        else:
                            lhsT = wb[b:b + K, woff:woff + 64]
                            rhs = src[b:b + K, p, sl]
                            pos = (b, b)
                        nc.tensor.matmul(pt[b:b + 64, sl], lhsT, rhs,
                                         start=True, stop=True,
                                         tile_position=pos,
                                         skip_group_check=True)
                    if dst is not None:
                        eng_i = (li + p) if n is None else n
                        epilogue(eng_i, dst[:, p, sl], pt[:, sl],
                                 bb[:, bcol:bcol + 1])
                    else:
                        if n in (None, 0):
                            s4osb[p] = opool.tile([128, NT], F32,
                                                  tag="osb2", name="osb2")
                        osb = s4osb[p]
                        if n == 1:
                            nc.vector.tensor_scalar_add(
                                osb[:65, sl], pt[:65, sl], bb[:65, 12:13])
                        else:
                            nc.scalar.activation(osb[:65, sl], pt[:65, sl],
                                                 Ident, bias=bb[:65, 12:13])
                        if n != 0:
                            if outmerge:
                                out_eng.dma_start(
                                    out_ext[0, 2 * p * NT:(2 * p + 2) * NT]
                                    .rearrange("(s j) -> s j", s=2),
                                    osb[0:65:64, :])
                            else:
                                for s in (0, 1):
                                    t = 2 * p + s
                                    out_eng.dma_start(
                                        out_ext[0:1, t * NT:(t + 1) * NT],
                                        osb[64 * s:64 * s + 1, :])
                return do_step

            def body(prev=None, defer_tail=False):
                h0s, h1s = {}, {}
                h2all = apool.tile([128, NTILES, NT], BF, tag="h2all")
                do_step = make_tail(h2all) if mode == "full" else None
                if warmup:
                    # keep the PE busy through the x0 DMA wait so the HAM
                    # clock gate stays at 8/8 across the iteration boundary
                    wu = ppb.tile([128, NT], F32, tag="pb", name="wu")
                    for _ in range(warmup):
                        nc.tensor.matmul(wu[0:32, 0:128], wl8[:, 960:992],
                                         wl8[:, 0:128], start=True,
                                         stop=True, tile_position=(0, 0),
                                         skip_group_check=True)
                first = True
                for s in range(NTILES + 2):
                    if prev is not None:
                        tin = {"two": TRAIL_IN, "spread": TRAIL_IN_SPREAD,
                               "late": TRAIL_IN_LATE}[trailin]
                        for p, li in tin.get(s, []):
                            prev(p, li)
                    if order == "v1":
                        if s < NTILES:
                            stage_local(s, h0s)
                        if last_stage >= 2 and 1 <= s <= NTILES:
                            stage_w1(s - 1, h0s, h1s)
                        if last_stage >= 3 and 2 <= s <= NTILES + 1:
                            t2 = s - 2
                            stage_w2(t2, h1s, h2all)
                            if do_step is not None:
                                for p, li in TAIL_AT.get(t2, []):
                                    do_step(p, li)
                    else:
                        if last_stage >= 3 and 2 <= s <= NTILES + 1:
                            t2 = s - 2
                            if order2 and do_step is not None:
                                for p, li in TAIL_AT.get(t2, []):
                                    do_step(p, li)
                                stage_w2(t2, h1s, h2all)
                            else:
                                stage_w2(t2, h1s, h2all)
                                if do_step is not None:
                                    for p, li in TAIL_AT.get(t2, []):
                                        do_step(p, li)
                        if last_stage >= 2 and 1 <= s <= NTILES:
                            stage_w1(s - 1, h0s, h1s)
                        if s < NTILES:
                            stage_local(s, h0s)
                if do_step is not None and not defer_tail:
                    if half_trail:
                        for p, li, n in TAIL_TRAILING_HALF:
                            do_step(p, li, n)
                    else:
                        for p, li in TAIL_TRAILING:
                            do_step(p, li)
                if mode == "notail":
                    out_probe(0, h2all[:1, 0, :])
                return do_step

            if w1slim and last_stage >= 2:
                # one-time zero of the h1p1 bank's pad rows; the slim mo=1
                # matmuls write only [0:91], rows 91:128 keep these zeros
                # (start=True clears has_written bits, not data)
                h1p1z = pp1.tile([128, NT], F32, tag="h1p1", name="h1p1z")
                nc.tensor.matmul(h1p1z[64:128, :], wl8[:, 960:1024],
                                 wl8[:, 0:NT], start=True, stop=True,
                                 tile_position=(0, 64),
                                 skip_group_check=True)
            if nodummy and last_stage >= 1:
                # one-time zero of the cb=3 bank's pad rows; start=True on
                # later matmuls clears has_written bits, not data, so these
                # zeros persist for every tile's cb=3 epilogue read
                h0p3z = pp0.tile([128, NT], F32, tag="h0p3", name="h0p3z")
                nc.tensor.matmul(h0p3z[96:128, :], wl8[:, 960:992],
                                 wl8[:, 0:NT], start=True, stop=True,
                                 tile_position=(0, 96),
                                 skip_group_check=True)
            if hw_loop and repeat > 1:
                un = unroll2 if isinstance(unroll2, int) else 1
                un = 2 if unroll2 is True else un
                while un > 1 and repeat % un:
                    un //= 2
                if un > 1:
                    with tc.For_i(0, repeat // un, 1, staggered_reset=stag):
                        prev = None
                        for i in range(un):
                            prev = body(prev,
                                        defer_tail=(defer
                                                    and mode == "full"
                                                    and i < un - 1))
                else:
                    with tc.For_i(0, repeat, 1, staggered_reset=stag):
                        body()
            else:
                for _ in range(repeat):
                    body()

    nc.finalize()
    return nc


_nc_cache = {}


def _get_nc():
    if "nc" not in _nc_cache:
        _nc_cache["nc"] = build_nc()
    return _nc_cache["nc"]


def prepare_in_maps(inputs):
    x = np.asarray(inputs["x"])
    wb = pack_weights(*(np.asarray(inputs[k]) for k in
                        ["W3", "W4", "Ws1", "Ws2", "Ws3", "Ws4"]))
    bb = pack_biases(*(np.asarray(inputs[k]) for k in
                       ["b_local", "b1", "b2", "b3", "b4",
                        "bs1", "bs2", "bs3", "bs4"]))
    wl8 = pack_local_fp8(np.asarray(inputs["W_local"]))
    wd1 = pack_w1_dr(np.asarray(inputs["W1"]))
    wd2 = pack_w2_dr(np.asarray(inputs["W2"]))
    in_maps = []
    for i in range(NCORES):
        xq = pack_x_shard(x[i * BC:(i + 1) * BC])
        in_maps.append({"xq": xq, "wb": wb, "bb": bb, "wl8": wl8,
                        "wd1": wd1, "wd2": wd2})
    return in_maps


def kernel(**inputs) -> np.ndarray:
    nc = _get_nc()
    in_maps = prepare_in_maps(inputs)
    res = run_bass_kernel_spmd(nc, in_maps, core_ids=list(range(NCORES)))
    out = np.concatenate([res.results[i]["out"].reshape(-1)
                          for i in range(NCORES)])
    return out.reshape(BATCH, 1).astype(np.float32)



# revision 69
# speedup vs baseline: 1.0134x; 1.0134x over previous
"""Trainium2 Bass kernel for nn_Architecture_59760174956735 (dense_mlp).

Pure data parallel over 8 NeuronCores: batch 32768 -> 8 x 4096 rows,
weights replicated; no collectives. Host prep transposes x to
feature-major fp8-e4m3 (f = z*16 + c as [partition, tile, chunk, col]),
so no on-chip transpose is needed and the x DMA is 8.4 MB/core.

Per core, a software pipeline over 8 N-tiles of 512 batch columns:
step s issues the locally-connected layer for tile s (fp8, 31 M=32
matmuls 4-way col-rotated via tile_position), W1 for tile s-1 and W2
for tile s-2 (both fp8 DoubleRow: K packed 2/cell, rhs [128,2,N] is the
natural chunk-pair layout), plus interleaved pair-tail steps computing
the five tiny tail layers for two tiles packed per PSUM bank at
partition bases 0/64 (tail weights zero-padded to M=64). PSUM->SBUF
epilogues fuse bias+ReLU, alternating Activation / Vector engines.
The packed tail-weight tensor carries only W3..Ws4 (98 KB) and weight
DMAs are ordered by first use so the x stream starts early; each
pair's two output rows go out as one strided DMA on the Activation
HWDGE ring, and x-tile DMAs alternate between the sync and Activation
rings so both HWDGE queue rows feed the SDMA engines; the repeat
loop uses For_i(staggered_reset=True) with sixteen body unrolls per
loop iteration, so the all-engine barrier fires every sixteenth pass; each
pass's eleven trailing tail steps are deferred into the NEXT pass's
early steps (two per step, drained first), so their latency-bound
chain links never park at the PE queue head in front of the next
pass's ready matmuls (-6.7us, the largest single win). The
cb=3 PSUM bank's pad rows (96:128) are zeroed once before the loop
instead of by a per-tile dummy matmul, and W1's second output group is
sliced to its real 91 columns with the h1p1 bank's pad rows pre-zeroed
the same way (start=True clears has_written bits, not data, so the
zeros persist for every epilogue read). Within each step, ready work
(W2, tail, W1) is emitted before the DMA-dependent local matmuls so
pass-boundary DMA waits never block the PE queue head.
Quantization error of the fp8
inputs/weights averages out through the deep contraction; measured
output rel err vs the f32 reference ~6.6e-4.
"""
import numpy as np
import ml_dtypes
from contextlib import ExitStack

from concourse import bacc, tile, mybir
from concourse.bass_utils import run_bass_kernel_spmd

BF16 = ml_dtypes.bfloat16
FP8 = ml_dtypes.float8_e4m3

BF = mybir.dt.bfloat16
F8 = mybir.dt.float8e4
F32 = mybir.dt.float32
Relu = mybir.ActivationFunctionType.Relu
Ident = mybir.ActivationFunctionType.Identity
ADD = mybir.AluOpType.add
MAX = mybir.AluOpType.max

NCORES = 8
BATCH = 32768
BC = BATCH // NCORES
NT = 512
NTILES = BC // NT

L, NHF, F1, S1_, NCH, NZ = 15, 32, 16, 8, 16, 128
H1, H2, H3, NF = 219, 100, 45, 21


L, NHF, F1, S1, NCH, NZ = 15, 32, 16, 8, 16, 128
H1, H2, H3, NF = 219, 100, 45, 21

# column offsets inside the packed tail-weight array wb [128, 384].
# Tail layers are zero-padded to M=64 so pair-packed matmuls write the full
# 64-partition range (epilogues may then read [0:128) without stale bytes).
OFF_W3 = 0            # 45 used of 64
OFF_W4 = 64           # 21 used of 64
OFF_S1 = 128          # 20 used of 64
OFF_S2 = 192          # 20 used of 64
OFF_S3 = 256          # 20 used of 64
OFF_S4 = 320          # 1 used of 64
WB_COLS = 384


def pack_x_shard(xs: np.ndarray, group: int = 512, dtype=FP8) -> np.ndarray:
    """(Bc, 16, 128) f32 -> xq2[p, g, q, j] where feature f = 128q + p
    (f = z*16+c) and batch b = g*group + j. Per-partition data for one batch
    group is contiguous for descriptor-efficient DMA."""
    Bc = xs.shape[0]
    xt = xs.astype(dtype).transpose(2, 1, 0)           # [z, c, b]
    xt = xt.reshape(16, 8, NCH, Bc)                    # [q, dz, c, b]
    xq = xt.transpose(1, 2, 0, 3).reshape(128, 16, Bc)  # [p, q, b]
    xq2 = xq.reshape(128, 16, Bc // group, group).transpose(0, 2, 1, 3)
    return np.ascontiguousarray(xq2)                   # [p, g, q, j]


def pack_local_fp8(W_local) -> np.ndarray:
    """Local-layer weights in fp8-e4m3: wl8[p, (2l+m)*32+o] = Wt[l, 128m+p, o]."""
    T = W_local.reshape(L, NHF, NCH, F1)               # [l, o, c, k]
    Wt = T.transpose(0, 3, 2, 1).reshape(L, 256, NHF)  # [l, k*16+c, o]
    wl = Wt.reshape(L, 2, 128, NHF).transpose(2, 0, 1, 3).reshape(128, 960)
    out = np.zeros((128, 1024), np.float32)  # cols 960:1024 all-zero block
    out[:, :960] = wl
    return out.astype(FP8)


def pack_weights(W3, W4, Ws1, Ws2, Ws3, Ws4) -> np.ndarray:
    wb = np.zeros((128, WB_COLS), dtype=np.float32)
    wb[:100, OFF_W3:OFF_W3 + H3] = W3.T
    # tail weights replicated at partition base 64 for position-rotated MMs
    for base in (0, 64):
        wb[base:base + H3, OFF_W4:OFF_W4 + 21] = W4.T
        wb[base:base + 21, OFF_S1:OFF_S1 + 20] = Ws1.T
        wb[base:base + 20, OFF_S2:OFF_S2 + 20] = Ws2.T
        wb[base:base + 20, OFF_S3:OFF_S3 + 20] = Ws3.T
        wb[base:base + 20, OFF_S4:OFF_S4 + 1] = Ws4.T
    return wb.astype(BF16)


def pack_biases(b_local, b1, b2, b3, b4, bs1, bs2, bs3, bs4) -> np.ndarray:
    bb = np.zeros((128, 13), dtype=np.float32)
    bl = b_local.reshape(480)
    for c in range(4):
        n = min(128, 480 - c * 128)
        bb[:n, c] = bl[c * 128:c * 128 + n]
    bb[:128, 4] = b1[:128]
    bb[:91, 5] = b1[128:]
    bb[:100, 6] = b2
    for base in (0, 64):
        bb[base:base + H3, 7] = b3
        bb[base:base + 21, 8] = b4
        bb[base:base + 20, 9] = bs1
        bb[base:base + 20, 10] = bs2
        bb[base:base + 20, 11] = bs3
        bb[base:base + 1, 12] = bs4
    return bb


def pack_w1_dr(W1) -> np.ndarray:
    """W1 for fp8 DoubleRow: wd1[p, pair, i, o] = W1T_pad[128*(2*pair+i)+p, o],
    M padded to 224 so the i-stride (224 B) is 16-aligned."""
    w1t = np.zeros((512, 256), np.float32)
    w1t[:480, :H1] = W1.T
    return np.ascontiguousarray(
        w1t.reshape(2, 2, 128, 256).transpose(2, 0, 1, 3)).astype(FP8)


def pack_w2_dr(W2) -> np.ndarray:
    """W2 for fp8 DoubleRow: wd2[p, i, o] = W2T_pad[128*i+p, o], M pad 112."""
    w2t = np.zeros((256, 112), np.float32)
    w2t[:219, :H2] = W2.T
    return np.ascontiguousarray(
        w2t.reshape(2, 128, 112).transpose(1, 0, 2)).astype(FP8)


STAGES = {"dma": 0, "local": 1, "w1": 2, "w2": 3, "notail": 3, "full": 3}

# tail steps emitted after W2 of tile t2 (pair p ready once t2 >= 2p+1)
TAIL_AT = {2: [(0, 0)], 3: [(0, 1)],
           4: [(0, 2), (1, 0)], 5: [(0, 3), (1, 1)],
           6: [(0, 4), (1, 2), (2, 0)],
           7: [(0, 5), (1, 3), (2, 1), (3, 0)]}
TAIL_TRAILING = [(1, 4), (2, 2), (3, 1), (1, 5), (2, 3), (3, 2),
                 (2, 4), (3, 3), (2, 5), (3, 4), (3, 5)]
# previous pass's trailing steps drained inside the next pass, oldest
# first, two per main step (chain links get a full step of epi slack)
TRAIL_IN = {0: [(1, 4), (2, 2)], 1: [(3, 1), (1, 5)],
            2: [(2, 3), (3, 2)], 3: [(2, 4), (3, 3)],
            4: [(2, 5), (3, 4)], 5: [(3, 5)]}
# alternative placements of the deferred steps
TRAIL_IN_SPREAD = {0: [(1, 4)], 1: [(2, 2)], 2: [(3, 1)], 3: [(1, 5)],
                   4: [(2, 3)], 5: [(3, 2)], 6: [(2, 4)], 7: [(3, 3)],
                   8: [(2, 5), (3, 4)], 9: [(3, 5)]}
TRAIL_IN_LATE = {2: [(1, 4), (2, 2)], 3: [(3, 1), (1, 5)],
                 4: [(2, 3), (3, 2)], 5: [(2, 4), (3, 3)],
                 6: [(2, 5), (3, 4)], 7: [(3, 5)]}
# half-N (256-col) trailing: deep chains of pairs 2/3 advance in half
# steps on alternating engines, roughly halving per-link latency
TAIL_TRAILING_HALF = [
    (1, 4, 0), (1, 4, 1), (2, 2, 0), (2, 2, 1), (3, 1, 0), (3, 1, 1),
    (1, 5, 0), (1, 5, 1), (2, 3, 0), (2, 3, 1), (3, 2, 0), (3, 2, 1),
    (2, 4, 0), (2, 4, 1), (3, 3, 0), (3, 3, 1),
    (2, 5, 0), (2, 5, 1), (3, 4, 0), (3, 4, 1),
    (3, 5, 0), (3, 5, 1),
]


def build_nc(repeat=1, hw_loop=False, mode="full", out_ring="scalar",
             x0split=False, order="rev", half_trail=False, warmup=0,
             xbufs=3, stag=True, outmerge=True, unroll2=16,
             nodummy=True, w1slim=True, xring2=True, order2=False,
             defer=True, trailin="two", w1mix=False, hbufs=2,
             obufs=4):
    last_stage = STAGES[mode]
    nc = bacc.Bacc(None, target_bir_lowering=False)
    xq_ext = nc.declare_dram_parameter(
        "xq", [128, NTILES, 16, NT], F8, isOutput=False)
    wb_ext = nc.declare_dram_parameter("wb", [128, WB_COLS], BF, isOutput=False)
    wl8_ext = nc.declare_dram_parameter("wl8", [128, 1024], F8, isOutput=False)
    wd1_ext = nc.declare_dram_parameter("wd1", [128, 2, 2, 256], F8,
                                        isOutput=False)
    wd2_ext = nc.declare_dram_parameter("wd2", [128, 2, 112], F8,
                                        isOutput=False)
    bb_ext = nc.declare_dram_parameter("bb", [128, 13], F32, isOutput=False)
    out_ext = nc.declare_dram_parameter("out", [1, BC], F32, isOutput=True)

    with tile.TileContext(nc) as tc, ExitStack() as ctx:
        wpool = ctx.enter_context(tc.tile_pool(name="w", bufs=1))
        xpool = ctx.enter_context(tc.tile_pool(name="x", bufs=xbufs))
        hpool = ctx.enter_context(tc.tile_pool(name="h", bufs=hbufs))
        apool = ctx.enter_context(tc.tile_pool(name="a", bufs=1))
        opool = ctx.enter_context(tc.tile_pool(name="o", bufs=obufs))

        wb = wpool.tile([128, WB_COLS], BF, tag="wb")
        bb = wpool.tile([128, 13], F32, tag="bb")
        wl8 = wpool.tile([128, 1024], F8, tag="wl8")
        wd1 = wpool.tile([128, 2, 2, 256], F8, tag="wd1")
        wd2 = wpool.tile([128, 2, 112], F8, tag="wd2")
        # order by first use: local weights, then W1/W2, then tail
        nc.sync.dma_start(wl8[:], wl8_ext[:])
        nc.sync.dma_start(bb[:], bb_ext[:])
        nc.sync.dma_start(wd1[:], wd1_ext[:])
        nc.sync.dma_start(wd2[:], wd2_ext[:])
        nc.sync.dma_start(wb[:], wb_ext[:])

        def epilogue(i, out_ap, in_ap, bias_ap, relu=True):
            if not relu:
                nc.scalar.activation(out_ap, in_ap, Ident, bias=bias_ap)
            elif i % 2 == 0:
                nc.scalar.activation(out_ap, in_ap, Relu, bias=bias_ap)
            else:
                nc.vector.tensor_scalar(out_ap, in_ap, bias_ap, 0.0,
                                        op0=ADD, op1=MAX)

        out_eng = nc.sync if out_ring == "sync" else nc.scalar

        def out_probe(t, src_ap):
            """Stripped-mode output: 1-row copy + DMA so work stays live."""
            osb = opool.tile([1, NT], F32, tag="osb")
            nc.vector.tensor_copy(osb[:1, :], src_ap)
            out_eng.dma_start(out_ext[0:1, t * NT:(t + 1) * NT], osb[:1, :])

        with tc.tile_pool(name="p0", bufs=1, space="PSUM") as pp0, \
             tc.tile_pool(name="p1", bufs=1, space="PSUM") as pp1, \
             tc.tile_pool(name="pb", bufs=2, space="PSUM") as ppb:

            def stage_local(t, h0s):
                xsb = xpool.tile([128, 16, NT], F8, tag="xsb")
                xeng = nc.scalar if (xring2 and t % 2) else nc.sync
                if x0split and t == 0:
                    for lo, hi in ((0, 5), (5, 9), (9, 13), (13, 16)):
                        xeng.dma_start(xsb[:, lo:hi, :],
                                       xq_ext[:, 0, lo:hi, :])
                else:
                    xeng.dma_start(xsb[:], xq_ext[:, t, :, :])
                if last_stage == 0:
                    out_probe(t, xsb[:1, 0, :])
                    return
                h0 = hpool.tile([128, 4, NT], F8, tag="h0")
                w1g = h0s.pop("w1g", None)
                for cb in range(4):
                    h0p = pp0.tile([128, NT], F32, tag=f"h0p{cb}")
                    nblk = 4 if cb < 3 else 3
                    if cb == 3 and not nodummy:
                        # zero dummy block so rows 96:128 are written
                        nc.tensor.matmul(h0p[96:128, :], wl8[:, 960:992],
                                         xsb[:, 15, :], start=True, stop=True,
                                         tile_position=(0, 96),
                                         skip_group_check=True)
                    for m in (0, 1):
                        for i in range(nblk):
                            l = cb * 4 + i
                            po = 32 * i
                            nc.tensor.matmul(
                                h0p[po:po + 32, :],
                                wl8[:, (2 * l + m) * 32:
                                       (2 * l + m + 1) * 32],
                                xsb[:, l + m, :],
                                start=(m == 0), stop=(m == 1),
                                tile_position=(0, po),
                                skip_group_check=True,
                            )
                    if w1g is not None:
                        next(w1g, None)
                    epilogue(t + cb, h0[:, cb, :], h0p[:, :],
                             bb[:, cb:cb + 1])
                h0s[t] = h0
                if w1g is not None:
                    for _ in w1g:
                        pass
                if last_stage == 1:
                    out_probe(t, h0[:1, 0, :])

            def w1_pieces(t, h0s, h1s):
                h0 = h0s.pop(t)
                h1 = hpool.tile([128, 2, NT], F8, tag="h1", name="h1")
                for mo in range(2):
                    h1p = pp1.tile([128, NT], F32, tag=f"h1p{mo}",
                                   name="h1p")
                    mw = 91 if (w1slim and mo == 1) else 128
                    for pair in (0, 1):
                        nc.tensor.matmul(
                            h1p[:mw, :],
                            wd1[:, pair, :, 128 * mo:128 * mo + mw],
                            h0[:, 2 * pair:2 * pair + 2, :],
                            start=(pair == 0), stop=(pair == 1),
                            perf_mode=mybir.MatmulPerfMode.DoubleRow,
                            skip_group_check=True,
                        )
                        yield
                    epilogue(t + mo, h1[:, mo, :], h1p[:, :],
                             bb[:, 4 + mo:5 + mo])
                h1s[t] = h1
                if last_stage == 2:
                    out_probe(t, h1[:1, 0, :])

            def stage_w1(t, h0s, h1s):
                for _ in w1_pieces(t, h0s, h1s):
                    pass

            def stage_w2(t, h1s, h2all):
                h1 = h1s.pop(t)
                h2p = ppb.tile([128, NT], F32, tag="pb")
                nc.tensor.matmul(
                    h2p[:100, :],
                    wd2[:, :, :100],
                    h1[:, 0:2, :],
                    start=True, stop=True,
                    perf_mode=mybir.MatmulPerfMode.DoubleRow,
                )
                epilogue(t, h2all[:100, t, :], h2p[:100, :], bb[:100, 6:7])
                if mode == "w2":
                    out_probe(t, h2all[:1, t, :])

            def make_tail(h2all):
                h3a = apool.tile([128, 4, NT], BF, tag="h3a")
                h4a = apool.tile([128, 4, NT], BF, tag="h4a")
                s1a = apool.tile([128, 4, NT], BF, tag="s1a")
                s2a = apool.tile([128, 4, NT], BF, tag="s2a")
                s3a = apool.tile([128, 4, NT], BF, tag="s3a")
                chain = [(None, h3a, 100, OFF_W3, 7),
                         (h3a, h4a, 45, OFF_W4, 8),
                         (h4a, s1a, 21, OFF_S1, 9),
                         (s1a, s2a, 20, OFF_S2, 10),
                         (s2a, s3a, 20, OFF_S3, 11),
                         (s3a, None, 20, OFF_S4, 12)]

                s4osb = {}

                def do_step(p, li, n=None):
                    src, dst, K, woff, bcol = chain[li]
                    sl = (slice(0, NT) if n is None
                          else slice(n * 256, (n + 1) * 256))
                    pt = ppb.tile([128, NT], F32, tag="pb")
                    for s in (0, 1):
                        b = 64 * s
                        if src is None:
                            lhsT = wb[:100, woff:woff + 64]
                            rhs = h2all[:100, 2 * p + s, sl]
                            pos = (0, b)
                        else:
                            lhsT = wb[b:b + K, woff:woff + 64]
                            rhs = src[b:b + K, p, sl]
                            pos = (b, b)
                        nc.tensor.matmul(pt[b:b + 64, sl], lhsT, rhs,
                                         start=True, stop=True,
                                         tile_position=pos,
                                         skip_group_check=True)
                    if dst is not None:
                        eng_i = (li + p) if n is None else n
                        epilogue(eng_i, dst[:, p, sl], pt[:, sl],
                                 bb[:, bcol:bcol + 1])
                    else:
                        if n in (None, 0):
                            s4osb[p] = opool.tile([128, NT], F32,
                                                  tag="osb2", name="osb2")
                        osb = s4osb[p]
                        if n == 1:
                            nc.vector.tensor_scalar_add(
                                osb[:65, sl], pt[:65, sl], bb[:65, 12:13])
                        else:
                            nc.scalar.activation(osb[:65, sl], pt[:65, sl],
                                                 Ident, bias=bb[:65, 12:13])
                        if n != 0:
                            if outmerge:
                                out_eng.dma_start(
                                    out_ext[0, 2 * p * NT:(2 * p + 2) * NT]
                                    .rearrange("(s j) -> s j", s=2),
                                    osb[0:65:64, :])
                            else:
                                for s in (0, 1):
                                    t = 2 * p + s
                                    out_eng.dma_start(
                                        out_ext[0:1, t * NT:(t + 1) * NT],
                                        osb[64 * s:64 * s + 1, :])
                return do_step

            def body(prev=None, defer_tail=False):
                h0s, h1s = {}, {}
                h2all = apool.tile([128, NTILES, NT], BF, tag="h2all")
                do_step = make_tail(h2all) if mode == "full" else None
                if warmup:
                    # keep the PE busy through the x0 DMA wait so the HAM
                    # clock gate stays at 8/8 across the iteration boundary
                    wu = ppb.tile([128, NT], F32, tag="pb", name="wu")
                    for _ in range(warmup):
                        nc.tensor.matmul(wu[0:32, 0:128], wl8[:, 960:992],
                                         wl8[:, 0:128], start=True,
                                         stop=True, tile_position=(0, 0),
                                         skip_group_check=True)
                first = True
                for s in range(NTILES + 2):
                    if prev is not None:
                        tin = {"two": TRAIL_IN, "spread": TRAIL_IN_SPREAD,
                               "late": TRAIL_IN_LATE}[trailin]
                        for p, li in tin.get(s, []):
                            prev(p, li)
                    if order == "v1":
                        if s < NTILES:
                            stage_local(s, h0s)
                        if last_stage >= 2 and 1 <= s <= NTILES:
                            stage_w1(s - 1, h0s, h1s)
                        if last_stage >= 3 and 2 <= s <= NTILES + 1:
                            t2 = s - 2
                            stage_w2(t2, h1s, h2all)
                            if do_step is not None:
                                for p, li in TAIL_AT.get(t2, []):
                                    do_step(p, li)
                    else:
                        if last_stage >= 3 and 2 <= s <= NTILES + 1:
                            t2 = s - 2
                            if order2 and do_step is not None:
                                for p, li in TAIL_AT.get(t2, []):
                                    do_step(p, li)
                                stage_w2(t2, h1s, h2all)
                            else:
                                stage_w2(t2, h1s, h2all)
                                if do_step is not None:
                                    for p, li in TAIL_AT.get(t2, []):
                                        do_step(p, li)
                        if w1mix and last_stage >= 2:
                            if 1 <= s < NTILES:
                                h0s["w1g"] = w1_pieces(s - 1, h0s, h1s)
                            elif s == NTILES:
                                stage_w1(s - 1, h0s, h1s)
                        elif last_stage >= 2 and 1 <= s <= NTILES:
                            stage_w1(s - 1, h0s, h1s)
                        if s < NTILES:
                            stage_local(s, h0s)
                if do_step is not None and not defer_tail:
                    if half_trail:
                        for p, li, n in TAIL_TRAILING_HALF:
                            do_step(p, li, n)
                    else:
                        for p, li in TAIL_TRAILING:
                            do_step(p, li)
                if mode == "notail":
                    out_probe(0, h2all[:1, 0, :])
                return do_step

            if w1slim and last_stage >= 2:
                # one-time zero of the h1p1 bank's pad rows; the slim mo=1
                # matmuls write only [0:91], rows 91:128 keep these zeros
                # (start=True clears has_written bits, not data)
                h1p1z = pp1.tile([128, NT], F32, tag="h1p1", name="h1p1z")
                nc.tensor.matmul(h1p1z[64:128, :], wl8[:, 960:1024],
                                 wl8[:, 0:NT], start=True, stop=True,
                                 tile_position=(0, 64),
                                 skip_group_check=True)
            if nodummy and last_stage >= 1:
                # one-time zero of the cb=3 bank's pad rows; start=True on
                # later matmuls clears has_written bits, not data, so these
                # zeros persist for every tile's cb=3 epilogue read
                h0p3z = pp0.tile([128, NT], F32, tag="h0p3", name="h0p3z")
                nc.tensor.matmul(h0p3z[96:128, :], wl8[:, 960:992],
                                 wl8[:, 0:NT], start=True, stop=True,
                                 tile_position=(0, 96),
                                 skip_group_check=True)
            if hw_loop and repeat > 1:
                un = unroll2 if isinstance(unroll2, int) else 1
                un = 2 if unroll2 is True else un
                while un > 1 and repeat % un:
                    un //= 2
                if un > 1:
                    with tc.For_i(0, repeat // un, 1, staggered_reset=stag):
                        prev = None
                        for i in range(un):
                            prev = body(prev,
                                        defer_tail=(defer
                                                    and mode == "full"
                                                    and i < un - 1))
                else:
                    with tc.For_i(0, repeat, 1, staggered_reset=stag):
                        body()
            else:
                for _ in range(repeat):
                    body()

    nc.finalize()
    return nc


_nc_cache = {}


def _get_nc():
    if "nc" not in _nc_cache:
        _nc_cache["nc"] = build_nc()
    return _nc_cache["nc"]


def prepare_in_maps(inputs):
    x = np.asarray(inputs["x"])
    wb = pack_weights(*(np.asarray(inputs[k]) for k in
                        ["W3", "W4", "Ws1", "Ws2", "Ws3", "Ws4"]))
    bb = pack_biases(*(np.asarray(inputs[k]) for k in
                       ["b_local", "b1", "b2", "b3", "b4",
                        "bs1", "bs2", "bs3", "bs4"]))
    wl8 = pack_local_fp8(np.asarray(inputs["W_local"]))
    wd1 = pack_w1_dr(np.asarray(inputs["W1"]))
    wd2 = pack_w2_dr(np.asarray(inputs["W2"]))
    in_maps = []
    for i in range(NCORES):
        xq = pack_x_shard(x[i * BC:(i + 1) * BC])
        in_maps.append({"xq": xq, "wb": wb, "bb": bb, "wl8": wl8,
                        "wd1": wd1, "wd2": wd2})
    return in_maps


def kernel(**inputs) -> np.ndarray:
    nc = _get_nc()
    in_maps = prepare_in_maps(inputs)
    res = run_bass_kernel_spmd(nc, in_maps, core_ids=list(range(NCORES)))
    out = np.concatenate([res.results[i]["out"].reshape(-1)
                          for i in range(NCORES)])
    return out.reshape(BATCH, 1).astype(np.float32)

